# revision 26
# baseline (speedup 1.0000x reference)
"""Trainium2 Bass kernel for nn_Dense_1322849927863 (segment_reduce).

Reference computation:
  h   = einsum('bltf,l->btf', x, aggr_w)            # layer aggregation (L=12)
  h   = relu(h @ w1.T + b1)                         # [B,T,H=256]
  h   = relu(h @ w2.T + b2)                         # [B,T,256]
  pooled = (h * mask).sum(t) / lengths              # masked mean over t<len
  out = pooled @ w3.T + b3                          # [B,8]

Strategy (8 NeuronCores):
  - Even row split: the valid (b,t) rows of all 16 batches (sorted by
    descending length) are concatenated and split into 8 near-equal chunks,
    one per core.  A chunk may contain up to `smax` batch segments; host
    builds a per-t pooling weight matrix maskw[t, seg] = 1/len(b) (0 for
    padding), so pooling is a single PE matmul contraction over t.
  - x streams as fp8 e4m3 (TRN FP8_EXP4) halving the DMA roofline vs bf16.
  - Layer aggregation as fp8 DoubleRow matmuls: contraction packs TWO
    t-groups (K = 2x120 partition-pairs), so each of 6 pair-matmuls per
    384-col chunk streams x at 2 elem/lane/cycle — half the PE-streaming
    cost of the normal-mode formulation.
  - h transposed for mm1 via matmul-with-identity (stationary = hagg chunk,
    FWL-eligible 128-col bf16 weights) instead of tensor.transpose — ~5x
    cheaper per 128-col chunk.
  - mm1 batches the t (free) dim of 2 subtiles per stationary load of w1.
  - mm2 in [t, g] orientation (stationary = h2 chunk) with b2 added via a
    ones-row matmul, so pooling can contract t on partitions.  Classifier
    runs per-core WITHOUT b3 (partial sums for split batches are combined
    on host, then b3 added once).
  - All x subtile DMAs are issued up front on the SP (sync) HWDGE ring
    only (bufs=nsub; x fits in SBUF) with flat 2D APs so descriptors are
    one 9216B run per partition.  The ACT ring is kept free for compute —
    measured per-core HBM read tops out at ~220 GB/s here, so the kernel
    is DMA-bound and everything else must hide under it.
  - The last subtile ships only its valid 10-row t-groups (rounded to a
    DoubleRow pair); untouched agg output rows are written as clean zeros
    by the PE and nullified by the pooling mask.
  - Per-subtile PE order: [epilogue of the previous batch] then the
    sem-waiting agg matmuls, so ready work is never queued behind a DMA
    wait in the in-order PE queue.
"""

import numpy as np

B, L, T, F = 16, 12, 1024, 768
H, NL = 256, 8
NCORES = 8
P = 128
G = 10           # t-positions per aggregation group
SUB = 120        # t-rows per subtile (12 groups of 10)
FC = 384         # f columns per aggregation PSUM tile (2 chunks = 768)

_CACHE = {}
LAST_RESULTS = None   # BassKernelResults from the most recent run (for test.py)
LAST_META = None      # (tpad, smax, in_maps) from the most recent run
LAST_LGROUPS = 12     # last-subtile group count from the most recent run


def _build_bass(tpad, smax, reps=0, lgroups=12):
    import concourse.bass as bass
    import concourse.mybir as mybir
    import concourse.tile as tile
    from concourse import bacc

    f32 = mybir.dt.float32
    f32r = mybir.dt.float32r
    bf = mybir.dt.bfloat16
    f8 = mybir.dt.float8e4
    AF = mybir.ActivationFunctionType
    DR = mybir.MatmulPerfMode.DoubleRow

    nsub = tpad // SUB
    # subtile batches for mm1 (pairs; odd nsub gets a singleton tail)
    batches = [list(range(b, min(b + 2, nsub))) for b in range(0, nsub, 2)]
    sub2batch = {u: bi for bi, bs in enumerate(batches) for u in bs}

    nc = bacc.Bacc()
    x_h = nc.dram_tensor("x", [nsub, SUB, 12, F], f8, kind="ExternalInput")
    ag_h = nc.dram_tensor("ag2", [SUB, 12, P], f8, kind="ExternalInput")
    id_h = nc.dram_tensor("ident", [SUB, SUB], bf, kind="ExternalInput")
    w1_h = nc.dram_tensor("w1t", [P, 6, 2, P], bf, kind="ExternalInput")
    b1_h = nc.dram_tensor("b1s", [P, 2], f32, kind="ExternalInput")
    w2_h = nc.dram_tensor("w2t", [P, 2, H], bf, kind="ExternalInput")
    b2_h = nc.dram_tensor("b2r", [1, H], bf, kind="ExternalInput")
    on_h = nc.dram_tensor("ones", [1, SUB], bf, kind="ExternalInput")
    mk_h = nc.dram_tensor("maskw", [SUB, nsub, smax], f32r, kind="ExternalInput")
    w3_h = nc.dram_tensor("w3t", [P, 2, NL], f32r, kind="ExternalInput")
    ip_h = nc.dram_tensor("idp", [smax, smax], f32r, kind="ExternalInput")
    out_h = nc.dram_tensor("out", [NL, smax], f32, kind="ExternalOutput")

    with tile.TileContext(nc) as tc:
        with (
            tc.tile_pool(name="const", bufs=1) as const,
            tc.tile_pool(name="xp", bufs=nsub) as xp,
            tc.tile_pool(name="hg", bufs=3) as hg,
            tc.tile_pool(name="hT", bufs=3) as hTp,
            tc.tile_pool(name="h2p", bufs=2) as h2p,
            tc.tile_pool(name="h3p", bufs=2) as h3p,
            tc.tile_pool(name="fin", bufs=2) as fin,
            tc.tile_pool(name="psA", bufs=2, space="PSUM") as psA,
            tc.tile_pool(name="psT", bufs=2, space="PSUM") as psT,
            tc.tile_pool(name="ps1", bufs=1, space="PSUM") as ps1,
            tc.tile_pool(name="ps2", bufs=1, space="PSUM") as ps2,
            tc.tile_pool(name="psP", bufs=1, space="PSUM") as psP,
        ):
            # ---- constants into SBUF (ACT HWDGE ring; x uses the SP ring) ----
            ag_sb = const.tile([SUB, 12, P], f8)
            nc.scalar.dma_start(out=ag_sb, in_=ag_h[:, :, :])
            id_sb = const.tile([SUB, SUB], bf)
            nc.scalar.dma_start(out=id_sb, in_=id_h[:, :])
            w1_sb = const.tile([P, 6, 2, P], bf)
            nc.scalar.dma_start(out=w1_sb, in_=w1_h[:, :, :, :])
            b1_sb = const.tile([P, 2], f32)
            nc.scalar.dma_start(out=b1_sb, in_=b1_h[:, :])
            w2_sb = const.tile([P, 2, H], bf)
            nc.scalar.dma_start(out=w2_sb, in_=w2_h[:, :])
            b2_sb = const.tile([1, H], bf)
            nc.scalar.dma_start(out=b2_sb, in_=b2_h[:, :])
            on_sb = const.tile([1, SUB], bf)
            nc.scalar.dma_start(out=on_sb, in_=on_h[:, :])
            mk_sb = const.tile([SUB, nsub, smax], f32r)
            nc.scalar.dma_start(out=mk_sb, in_=mk_h[:, :, :])
            w3_sb = const.tile([P, 2, NL], f32r)
            nc.scalar.dma_start(out=w3_sb, in_=w3_h[:, :, :])
            ip_sb = const.tile([smax, smax], f32r)
            nc.scalar.dma_start(out=ip_sb, in_=ip_h[:, :])

            import contextlib
            rep_ctx = (tc.For_i(0, reps, 1, staggered_reset=True)
                           if reps else contextlib.nullcontext())
            with rep_ctx:
                poolp = psP.tile([smax, H], f32, tag="pool")

                hTs = {}   # batch idx -> haggT tile

                def epilogue(b):
                    us = batches[b]
                    tb = SUB * len(us)
                    haggT = hTs.pop(b)
                    mm1p = ps1.tile([P, 2, 2 * SUB], f32, tag="mm1")
                    h2 = h2p.tile([P, 2, 2 * SUB], bf, tag="h2")
                    for mh in range(2):
                        for kf in range(6):
                            nc.tensor.matmul(
                                mm1p[:, mh, 0:tb],
                                lhsT=w1_sb[:, kf, mh, :],
                                rhs=haggT[:, kf, 0:tb],
                                start=(kf == 0),
                                stop=(kf == 5),
                            )
                        nc.scalar.activation(
                            out=h2[:, mh, 0:tb],
                            in_=mm1p[:, mh, 0:tb],
                            func=AF.Relu,
                            bias=b1_sb[:, mh:mh + 1],
                            scale=1.0,
                        )
                    for si, u in enumerate(us):
                        mm2p = ps2.tile([SUB, H], f32, tag="mm2")
                        for kh in range(2):
                            nc.tensor.matmul(
                                mm2p,
                                lhsT=h2[:, kh, si * SUB:(si + 1) * SUB],
                                rhs=w2_sb[:, kh, :],
                                start=(kh == 0),
                                stop=False,
                            )
                        nc.tensor.matmul(
                            mm2p, lhsT=on_sb, rhs=b2_sb, start=False, stop=True,
                        )
                        h3 = h3p.tile([SUB, H], f32r, tag="h3")
                        nc.scalar.activation(
                            out=h3, in_=mm2p, func=AF.Relu, bias=0.0, scale=1.0,
                        )
                        nc.tensor.matmul(
                            poolp,
                            lhsT=mk_sb[:, u, :],
                            rhs=h3,
                            start=(u == 0),
                            stop=(u == nsub - 1),
                            skip_group_check=True,
                        )

                ready = []
                for u in range(nsub):
                    # the last subtile ships and aggregates only its valid
                    # 10-row t-groups (rounded up to a DoubleRow pair); the
                    # untouched output partitions are written as clean
                    # zeros by the PE and nullified by the pooling mask.
                    ng = 12 if u < nsub - 1 else lgroups
                    x_sb = xp.tile([SUB, 12, F], f8, tag="x")
                    # flat 2D view of the tile -> coalesced descriptors
                    x_flat = bass.AP(
                        x_sb.tensor, x_sb.offset,
                        [[12 * F, SUB], [1, ng * F]],
                    )
                    nc.sync.dma_start(
                        out=x_flat,
                        in_=bass.AP(
                            x_h, u * SUB * 12 * F,
                            [[12 * F, SUB], [1, ng * F]],
                        ),
                    )
                    # epilogue first: ready PE work sits AHEAD of the
                    # sem-waiting agg matmuls in the in-order PE queue
                    if ready:
                        epilogue(ready.pop(0))
                    # --- layer aggregation: fp8 DoubleRow, K = (gt,l) x 2 ---
                    aggA = psA.tile([P, FC], f32, tag="agg")
                    aggB = psA.tile([P, FC], f32, tag="agg")
                    for d in range(ng // 2):
                        for fc, agg in enumerate((aggA, aggB)):
                            nc.tensor.matmul(
                                agg,
                                lhsT=ag_sb[:, 2 * d:2 * d + 2, :],
                                rhs=x_sb[:, 2 * d:2 * d + 2,
                                         fc * FC:(fc + 1) * FC],
                                start=(d == 0),
                                stop=(d == ng // 2 - 1),
                                perf_mode=DR,
                            )
                    hagg = hg.tile([SUB, F], bf, tag="hagg")
                    nc.scalar.copy(out=hagg[:, 0:FC], in_=aggA[0:SUB, :])
                    nc.vector.tensor_copy(out=hagg[:, FC:F], in_=aggB[0:SUB, :])
                    # --- transpose h via matmul-with-identity (FWL) ---
                    bidx = sub2batch[u]
                    if bidx not in hTs:
                        haggT_new = hTp.tile([P, 6, 2 * SUB], bf, tag="hT")
                        hTs[bidx] = haggT_new
                    haggT = hTs[bidx]
                    toff = (u - batches[bidx][0]) * SUB
                    for half in range(2):
                        psTt = psT.tile([P, 3, SUB], f32, tag="psT")
                        for j in range(3):
                            k = half * 3 + j
                            nc.tensor.matmul(
                                psTt[:, j, :],
                                lhsT=hagg[:, k * P:(k + 1) * P],
                                rhs=id_sb,
                                start=True,
                                stop=True,
                            )
                        nc.vector.tensor_copy(
                            out=haggT[:, half * 3:half * 3 + 3,
                                      toff:toff + SUB],
                            in_=psTt,
                        )
                    if u == batches[bidx][-1]:
                        ready.append(bidx)
                while ready:
                    epilogue(ready.pop(0))

                # ---- finale: classifier on pooled partials (no b3) ----
                pooled = fin.tile([smax, H], f32r, tag="pooled")
                nc.scalar.copy(out=pooled, in_=poolp)
                psf = psT.tile([P, 2, smax], f32r, tag="psT")
                for kg in range(2):
                    nc.tensor.transpose(
                        psf[:, kg, :], pooled[:, kg * P:(kg + 1) * P], ip_sb,
                    )
                pooledT = fin.tile([P, 2, smax], f32r, tag="pooledT")
                nc.scalar.copy(out=pooledT, in_=psf)
                mm3p = ps2.tile([NL, smax], f32, tag="mm2")
                for kg in range(2):
                    nc.tensor.matmul(
                        mm3p,
                        lhsT=w3_sb[:, kg, :],
                        rhs=pooledT[:, kg, :],
                        start=(kg == 0),
                        stop=(kg == 1),
                    )
                o_sb = fin.tile([NL, smax], f32, tag="osb")
                nc.scalar.copy(out=o_sb, in_=mm3p)
                nc.sync.dma_start(out=out_h[:, :], in_=o_sb)
    nc.compile()
    return nc


def _prep_shared(aggr_w, w1, b1, w2, b2, w3):
    import ml_dtypes
    f8 = ml_dtypes.float8_e4m3
    bf = ml_dtypes.bfloat16
    ag2 = np.zeros((SUB, 12, P), dtype=np.float32)
    for i in range(12):
        for gt in range(G):
            for l in range(L):
                ag2[gt * L + l, i, i * G + gt] = aggr_w[l]
    w1t = np.ascontiguousarray(
        w1.T.reshape(6, P, 2, P).transpose(1, 0, 2, 3)).astype(bf)
    b1s = np.ascontiguousarray(b1.reshape(2, P).T).astype(np.float32)
    w2t = np.ascontiguousarray(
        w2.T.reshape(2, P, H).transpose(1, 0, 2)).astype(bf)
    b2r = b2.reshape(1, H).astype(bf)
    ones = np.ones((1, SUB), dtype=bf)
    w3t = np.ascontiguousarray(
        w3.T.reshape(2, P, NL).transpose(1, 0, 2)).astype(np.float32)
    ident = np.eye(SUB, dtype=np.float32).astype(bf)
    return {
        "ag2": ag2.astype(f8), "ident": ident, "w1t": w1t, "b1s": b1s,
        "w2t": w2t, "b2r": b2r, "ones": ones, "w3t": w3t,
    }


def _plan(lens):
    """Split the concatenated valid rows (batches sorted by descending
    length) into 8 near-equal chunks.  Returns (tpad, smax, segs) where
    segs[c] = list of (batch, src_t0, src_t1, dst_t0)."""
    order = np.argsort(-lens, kind="stable")
    total = int(lens.sum())
    R = -(-total // NCORES)
    tpad = max(SUB, -(-R // SUB) * SUB)
    segs = [[] for _ in range(NCORES)]
    c, used = 0, 0
    for b in order:
        b = int(b)
        rem, src = int(lens[b]), 0
        while rem > 0:
            cap = R if c < NCORES - 1 else total - (NCORES - 1) * R
            take = min(rem, cap - used)
            segs[c].append((b, src, src + take, used))
            used += take
            rem -= take
            src += take
            if used >= cap and c < NCORES - 1:
                c += 1
                used = 0
    smax = max(4, max(len(s) for s in segs))
    return tpad, smax, segs


def kernel(x, lengths, aggr_w, w1, b1, w2, b2, w3, b3):
    global LAST_RESULTS, LAST_META
    import ml_dtypes
    from concourse.bass_utils import run_bass_kernel_spmd
    f8 = ml_dtypes.float8_e4m3

    x = np.asarray(x, dtype=np.float32)
    lens = np.asarray(lengths).astype(np.int64)
    aggr_w = np.asarray(aggr_w, dtype=np.float32)
    w1 = np.asarray(w1, dtype=np.float32)
    b1 = np.asarray(b1, dtype=np.float32)
    w2 = np.asarray(w2, dtype=np.float32)
    b2 = np.asarray(b2, dtype=np.float32)
    w3 = np.asarray(w3, dtype=np.float32)
    b3 = np.asarray(b3, dtype=np.float32)

    tpad, smax, segs = _plan(lens)
    nsub = tpad // SUB
    # valid 10-row t-groups in the last subtile (rounded to a DR pair)
    R = -(-int(lens.sum()) // NCORES)
    lgroups = min(12, 2 * (-(-(R - (nsub - 1) * SUB) // 20)))

    key = (tpad, smax, lgroups, 0)
    if key not in _CACHE:
        _CACHE[key] = _build_bass(tpad, smax, lgroups=lgroups)
    nc = _CACHE[key]

    shared = _prep_shared(aggr_w, w1, b1, w2, b2, w3)
    shared["idp"] = np.eye(smax, dtype=np.float32)
    in_maps = []
    for c in range(NCORES):
        xt8 = np.zeros((12, tpad, F), dtype=f8)
        mkf = np.zeros((tpad, smax), dtype=np.float32)
        for j, (b, s0, s1, d0) in enumerate(segs[c]):
            n = s1 - s0
            xt8[:, d0:d0 + n] = x[b, :, s0:s1].astype(f8)
            mkf[d0:d0 + n, j] = 1.0 / float(lens[b])
        xq = np.ascontiguousarray(
            xt8.reshape(12, nsub, 12, G, F).transpose(1, 3, 0, 2, 4)
        ).reshape(nsub, SUB, 12, F)
        mk = np.ascontiguousarray(
            mkf.reshape(nsub, SUB, smax).transpose(1, 0, 2))
        in_maps.append({"x": xq, "maskw": mk, **shared})

    res = run_bass_kernel_spmd(nc, in_maps, core_ids=list(range(NCORES)))
    LAST_RESULTS = res
    LAST_META = (tpad, smax, in_maps)
    global LAST_LGROUPS
    LAST_LGROUPS = lgroups

    out = np.zeros((B, NL), dtype=np.float32)
    for c in range(NCORES):
        o = res.results[c]["out"]        # [NL, smax]
        for j, (b, s0, s1, d0) in enumerate(segs[c]):
            out[b] += o[:, j]
    out += b3[None, :]
    return out


# revision 35
# speedup vs baseline: 1.0309x; 1.0309x over previous
"""Trainium2 Bass kernel for nn_Dense_1322849927863 (segment_reduce).

Reference computation:
  h   = einsum('bltf,l->btf', x, aggr_w)            # layer aggregation (L=12)
  h   = relu(h @ w1.T + b1)                         # [B,T,H=256]
  h   = relu(h @ w2.T + b2)                         # [B,T,256]
  pooled = (h * mask).sum(t) / lengths              # masked mean over t<len
  out = pooled @ w3.T + b3                          # [B,8]

Strategy (8 NeuronCores):
  - Even row split: the valid (b,t) rows of all 16 batches (sorted by
    descending length) are concatenated and split into 8 near-equal chunks,
    one per core.  A chunk may contain up to `smax` batch segments; host
    builds a per-t pooling weight matrix maskw[t, seg] = 1/len(b) (0 for
    padding), so pooling is a single PE matmul contraction over t.
  - x streams as fp8 e4m3 (TRN FP8_EXP4) halving the DMA roofline vs bf16.
  - Layer aggregation as fp8 DoubleRow matmuls: contraction packs TWO
    t-groups (K = 2x120 partition-pairs), so each of 6 pair-matmuls per
    384-col chunk streams x at 2 elem/lane/cycle — half the PE-streaming
    cost of the normal-mode formulation.
  - h transposed for mm1 via matmul-with-identity (stationary = hagg chunk,
    FWL-eligible 128-col bf16 weights) instead of tensor.transpose — ~5x
    cheaper per 128-col chunk.
  - mm1 batches the t (free) dim of 2 subtiles per stationary load of w1.
  - mm2 in [t, g] orientation (stationary = h2 chunk) with b2 (a
    per-free-column bias) added on the DVE, off the PE critical path, so
    pooling can contract t on partitions.  Classifier runs per-core
    WITHOUT b3 (partial sums for split batches are combined on host,
    then b3 added once).
  - All x subtile DMAs are issued up front on the SP (sync) HWDGE ring
    only (bufs=nsub; x fits in SBUF) with flat 2D APs so descriptors are
    one 9216B run per partition.  The ACT ring is kept free for compute —
    measured per-core HBM read tops out at ~220 GB/s here, so the kernel
    is DMA-bound and everything else must hide under it.
  - The last subtile ships only its valid 10-row t-groups (rounded to a
    DoubleRow pair); untouched agg output rows are written as clean zeros
    by the PE and nullified by the pooling mask.
  - Per-subtile PE order: [epilogue of the previous batch] then the
    sem-waiting agg matmuls, so ready work is never queued behind a DMA
    wait in the in-order PE queue.
"""

import numpy as np

B, L, T, F = 16, 12, 1024, 768
H, NL = 256, 8
NCORES = 8
P = 128
G = 10           # t-positions per aggregation group
SUB = 120        # t-rows per subtile (12 groups of 10)
FC = 384         # f columns per aggregation PSUM tile (2 chunks = 768)

_CACHE = {}
LAST_RESULTS = None   # BassKernelResults from the most recent run (for test.py)
LAST_META = None      # (tpad, smax, in_maps) from the most recent run
LAST_LGROUPS = 12     # last-subtile group count from the most recent run


def _build_bass(tpad, smax, reps=0, lgroups=12):
    import concourse.bass as bass
    import concourse.mybir as mybir
    import concourse.tile as tile
    from concourse import bacc

    f32 = mybir.dt.float32
    f32r = mybir.dt.float32r
    bf = mybir.dt.bfloat16
    f8 = mybir.dt.float8e4
    AF = mybir.ActivationFunctionType
    DR = mybir.MatmulPerfMode.DoubleRow

    nsub = tpad // SUB
    # subtile batches for mm1 (pairs; odd nsub gets a singleton tail)
    batches = [list(range(b, min(b + 2, nsub))) for b in range(0, nsub, 2)]
    sub2batch = {u: bi for bi, bs in enumerate(batches) for u in bs}

    nc = bacc.Bacc()
    x_h = nc.dram_tensor("x", [nsub, SUB, 12, F], f8, kind="ExternalInput")
    ag_h = nc.dram_tensor("ag2", [SUB, 12, P], f8, kind="ExternalInput")
    id_h = nc.dram_tensor("ident", [SUB, SUB], bf, kind="ExternalInput")
    w1_h = nc.dram_tensor("w1t", [P, 6, 2, P], bf, kind="ExternalInput")
    b1_h = nc.dram_tensor("b1s", [P, 2], f32, kind="ExternalInput")
    w2_h = nc.dram_tensor("w2t", [P, 2, H], bf, kind="ExternalInput")
    b2_h = nc.dram_tensor("b2r", [SUB, H], bf, kind="ExternalInput")
    on_h = nc.dram_tensor("ones", [1, SUB], bf, kind="ExternalInput")
    mk_h = nc.dram_tensor("maskw", [SUB, nsub, smax], f32r, kind="ExternalInput")
    w3_h = nc.dram_tensor("w3t", [P, 2, NL], f32r, kind="ExternalInput")
    ip_h = nc.dram_tensor("idp", [smax, smax], f32r, kind="ExternalInput")
    out_h = nc.dram_tensor("out", [NL, smax], f32, kind="ExternalOutput")

    with tile.TileContext(nc) as tc:
        with (
            tc.tile_pool(name="const", bufs=1) as const,
            tc.tile_pool(name="xp", bufs=nsub) as xp,
            tc.tile_pool(name="hg", bufs=3) as hg,
            tc.tile_pool(name="hT", bufs=3) as hTp,
            tc.tile_pool(name="h2p", bufs=2) as h2p,
            tc.tile_pool(name="h3p", bufs=2) as h3p,
            tc.tile_pool(name="fin", bufs=2) as fin,
            tc.tile_pool(name="psA", bufs=2, space="PSUM") as psA,
            tc.tile_pool(name="psT", bufs=2, space="PSUM") as psT,
            tc.tile_pool(name="ps1", bufs=1, space="PSUM") as ps1,
            tc.tile_pool(name="ps2", bufs=1, space="PSUM") as ps2,
            tc.tile_pool(name="psP", bufs=1, space="PSUM") as psP,
        ):
            # ---- constants into SBUF (ACT HWDGE ring; x uses the SP ring) ----
            ag_sb = const.tile([SUB, 12, P], f8)
            nc.scalar.dma_start(out=ag_sb, in_=ag_h[:, :, :])
            id_sb = const.tile([SUB, SUB], bf)
            nc.scalar.dma_start(out=id_sb, in_=id_h[:, :])
            w1_sb = const.tile([P, 6, 2, P], bf)
            nc.scalar.dma_start(out=w1_sb, in_=w1_h[:, :, :, :])
            b1_sb = const.tile([P, 2], f32)
            nc.scalar.dma_start(out=b1_sb, in_=b1_h[:, :])
            w2_sb = const.tile([P, 2, H], bf)
            nc.scalar.dma_start(out=w2_sb, in_=w2_h[:, :])
            b2_sb = const.tile([SUB, H], bf)
            nc.scalar.dma_start(out=b2_sb, in_=b2_h[:, :])
            on_sb = const.tile([1, SUB], bf)
            nc.scalar.dma_start(out=on_sb, in_=on_h[:, :])
            mk_sb = const.tile([SUB, nsub, smax], f32r)
            nc.scalar.dma_start(out=mk_sb, in_=mk_h[:, :, :])
            w3_sb = const.tile([P, 2, NL], f32r)
            nc.scalar.dma_start(out=w3_sb, in_=w3_h[:, :, :])
            ip_sb = const.tile([smax, smax], f32r)
            nc.scalar.dma_start(out=ip_sb, in_=ip_h[:, :])
            # persistent output staging tile: in the looped (bench) build
            # each iteration's store is issued behind the NEXT set of x DMA
            # issues on the SP queue, hiding its ~2us HBM completion
            # receipt under the x stream instead of serializing it into
            # the loop barrier; a final store after the loop commits the
            # last result.
            o_res = const.tile([NL, smax], f32)
            nc.vector.memset(o_res, 0.0)

            import contextlib
            rep_ctx = (tc.For_i(0, reps, 1, staggered_reset=True)
                           if reps else contextlib.nullcontext())
            with rep_ctx:
                poolp = psP.tile([smax, H], f32, tag="pool")

                hTs = {}   # batch idx -> haggT tile

                def epilogue(b):
                    us = batches[b]
                    tb = SUB * len(us)
                    haggT = hTs.pop(b)
                    mm1p = ps1.tile([P, 2, 2 * SUB], f32, tag="mm1")
                    h2 = h2p.tile([P, 2, 2 * SUB], bf, tag="h2")
                    for mh in range(2):
                        for kf in range(6):
                            nc.tensor.matmul(
                                mm1p[:, mh, 0:tb],
                                lhsT=w1_sb[:, kf, mh, :],
                                rhs=haggT[:, kf, 0:tb],
                                start=(kf == 0),
                                stop=(kf == 5),
                            )
                        nc.scalar.activation(
                            out=h2[:, mh, 0:tb],
                            in_=mm1p[:, mh, 0:tb],
                            func=AF.Relu,
                            bias=b1_sb[:, mh:mh + 1],
                            scale=1.0,
                        )
                    for si, u in enumerate(us):
                        mm2p = ps2.tile([SUB, H], f32, tag="mm2")
                        for kh in range(2):
                            nc.tensor.matmul(
                                mm2p,
                                lhsT=h2[:, kh, si * SUB:(si + 1) * SUB],
                                rhs=w2_sb[:, kh, :],
                                start=(kh == 0),
                                stop=(kh == 1),
                            )
                        # b2 (a per-free-column bias, so not ACT-bias-able)
                        # added on DVE, off the PE critical path
                        h3a = h3p.tile([SUB, H], f32r, tag="h3a")
                        nc.vector.tensor_tensor(
                            out=h3a,
                            in0=mm2p,
                            in1=b2_sb,
                            op=mybir.AluOpType.add,
                        )
                        h3 = h3p.tile([SUB, H], f32r, tag="h3")
                        nc.scalar.activation(
                            out=h3, in_=h3a, func=AF.Relu, bias=0.0, scale=1.0,
                        )
                        nc.tensor.matmul(
                            poolp,
                            lhsT=mk_sb[:, u, :],
                            rhs=h3,
                            start=(u == 0),
                            stop=(u == nsub - 1),
                            skip_group_check=True,
                        )

                ready = []
                for u in range(nsub):
                    # the last subtile ships and aggregates only its valid
                    # 10-row t-groups (rounded up to a DoubleRow pair); the
                    # untouched output partitions are written as clean
                    # zeros by the PE and nullified by the pooling mask.
                    ng = 12 if u < nsub - 1 else lgroups
                    x_sb = xp.tile([SUB, 12, F], f8, tag="x")
                    # flat 2D view of the tile -> coalesced descriptors
                    x_flat = bass.AP(
                        x_sb.tensor, x_sb.offset,
                        [[12 * F, SUB], [1, ng * F]],
                    )
                    nc.sync.dma_start(
                        out=x_flat,
                        in_=bass.AP(
                            x_h, u * SUB * 12 * F,
                            [[12 * F, SUB], [1, ng * F]],
                        ),
                    )
                    # epilogue first: ready PE work sits AHEAD of the
                    # sem-waiting agg matmuls in the in-order PE queue
                    if ready:
                        epilogue(ready.pop(0))
                    # --- layer aggregation: fp8 DoubleRow, K = (gt,l) x 2 ---
                    aggA = psA.tile([P, FC], f32, tag="agg")
                    aggB = psA.tile([P, FC], f32, tag="agg")
                    for d in range(ng // 2):
                        for fc, agg in enumerate((aggA, aggB)):
                            nc.tensor.matmul(
                                agg,
                                lhsT=ag_sb[:, 2 * d:2 * d + 2, :],
                                rhs=x_sb[:, 2 * d:2 * d + 2,
                                         fc * FC:(fc + 1) * FC],
                                start=(d == 0),
                                stop=(d == ng // 2 - 1),
                                perf_mode=DR,
                            )
                    hagg = hg.tile([SUB, F], bf, tag="hagg")
                    nc.scalar.copy(out=hagg[:, 0:FC], in_=aggA[0:SUB, :])
                    nc.vector.tensor_copy(out=hagg[:, FC:F], in_=aggB[0:SUB, :])
                    # --- transpose h via matmul-with-identity (FWL) ---
                    bidx = sub2batch[u]
                    if bidx not in hTs:
                        haggT_new = hTp.tile([P, 6, 2 * SUB], bf, tag="hT")
                        hTs[bidx] = haggT_new
                    haggT = hTs[bidx]
                    toff = (u - batches[bidx][0]) * SUB
                    for half in range(2):
                        psTt = psT.tile([P, 3, SUB], f32, tag="psT")
                        for j in range(3):
                            k = half * 3 + j
                            nc.tensor.matmul(
                                psTt[:, j, :],
                                lhsT=hagg[:, k * P:(k + 1) * P],
                                rhs=id_sb,
                                start=True,
                                stop=True,
                            )
                        nc.vector.tensor_copy(
                            out=haggT[:, half * 3:half * 3 + 3,
                                      toff:toff + SUB],
                            in_=psTt,
                        )
                    if u == batches[bidx][-1]:
                        ready.append(bidx)
                    if u == nsub - 1 and reps:
                        # store the PREVIOUS iteration's result now: last in
                        # the SP queue behind all x DMA issues, so its ~2us
                        # completion receipt hides under the x stream
                        # instead of serializing into the loop barrier
                        nc.sync.dma_start(out=out_h[:, :], in_=o_res)
                while ready:
                    epilogue(ready.pop(0))

                # ---- finale: classifier on pooled partials (no b3) ----
                pooled = fin.tile([smax, H], f32r, tag="pooled")
                nc.scalar.copy(out=pooled, in_=poolp)
                psf = psT.tile([P, 2, smax], f32r, tag="psT")
                for kg in range(2):
                    nc.tensor.transpose(
                        psf[:, kg, :], pooled[:, kg * P:(kg + 1) * P], ip_sb,
                    )
                pooledT = fin.tile([P, 2, smax], f32r, tag="pooledT")
                nc.scalar.copy(out=pooledT, in_=psf)
                mm3p = ps2.tile([NL, smax], f32, tag="mm2")
                for kg in range(2):
                    nc.tensor.matmul(
                        mm3p,
                        lhsT=w3_sb[:, kg, :],
                        rhs=pooledT[:, kg, :],
                        start=(kg == 0),
                        stop=(kg == 1),
                    )
                nc.scalar.copy(out=o_res, in_=mm3p)
                if not reps:
                    nc.sync.dma_start(out=out_h[:, :], in_=o_res)
            if reps:
                nc.sync.dma_start(out=out_h[:, :], in_=o_res)
    nc.compile()
    return nc


def _prep_shared(aggr_w, w1, b1, w2, b2, w3):
    import ml_dtypes
    f8 = ml_dtypes.float8_e4m3
    bf = ml_dtypes.bfloat16
    ag2 = np.zeros((SUB, 12, P), dtype=np.float32)
    for i in range(12):
        for gt in range(G):
            for l in range(L):
                ag2[gt * L + l, i, i * G + gt] = aggr_w[l]
    w1t = np.ascontiguousarray(
        w1.T.reshape(6, P, 2, P).transpose(1, 0, 2, 3)).astype(bf)
    b1s = np.ascontiguousarray(b1.reshape(2, P).T).astype(np.float32)
    w2t = np.ascontiguousarray(
        w2.T.reshape(2, P, H).transpose(1, 0, 2)).astype(bf)
    b2r = np.ascontiguousarray(
        np.broadcast_to(b2.reshape(1, H), (SUB, H))).astype(bf)
    ones = np.ones((1, SUB), dtype=bf)
    w3t = np.ascontiguousarray(
        w3.T.reshape(2, P, NL).transpose(1, 0, 2)).astype(np.float32)
    ident = np.eye(SUB, dtype=np.float32).astype(bf)
    return {
        "ag2": ag2.astype(f8), "ident": ident, "w1t": w1t, "b1s": b1s,
        "w2t": w2t, "b2r": b2r, "ones": ones, "w3t": w3t,
    }


def _plan(lens):
    """Split the concatenated valid rows (batches sorted by descending
    length) into 8 near-equal chunks.  Returns (tpad, smax, segs) where
    segs[c] = list of (batch, src_t0, src_t1, dst_t0)."""
    order = np.argsort(-lens, kind="stable")
    total = int(lens.sum())
    R = -(-total // NCORES)
    tpad = max(SUB, -(-R // SUB) * SUB)
    segs = [[] for _ in range(NCORES)]
    c, used = 0, 0
    for b in order:
        b = int(b)
        rem, src = int(lens[b]), 0
        while rem > 0:
            cap = R if c < NCORES - 1 else total - (NCORES - 1) * R
            take = min(rem, cap - used)
            segs[c].append((b, src, src + take, used))
            used += take
            rem -= take
            src += take
            if used >= cap and c < NCORES - 1:
                c += 1
                used = 0
    smax = max(4, max(len(s) for s in segs))
    return tpad, smax, segs


def kernel(x, lengths, aggr_w, w1, b1, w2, b2, w3, b3):
    global LAST_RESULTS, LAST_META
    import ml_dtypes
    from concourse.bass_utils import run_bass_kernel_spmd
    f8 = ml_dtypes.float8_e4m3

    x = np.asarray(x, dtype=np.float32)
    lens = np.asarray(lengths).astype(np.int64)
    aggr_w = np.asarray(aggr_w, dtype=np.float32)
    w1 = np.asarray(w1, dtype=np.float32)
    b1 = np.asarray(b1, dtype=np.float32)
    w2 = np.asarray(w2, dtype=np.float32)
    b2 = np.asarray(b2, dtype=np.float32)
    w3 = np.asarray(w3, dtype=np.float32)
    b3 = np.asarray(b3, dtype=np.float32)

    tpad, smax, segs = _plan(lens)
    nsub = tpad // SUB
    # valid 10-row t-groups in the last subtile (rounded to a DR pair)
    R = -(-int(lens.sum()) // NCORES)
    lgroups = min(12, 2 * (-(-(R - (nsub - 1) * SUB) // 20)))

    key = (tpad, smax, lgroups, 0)
    if key not in _CACHE:
        _CACHE[key] = _build_bass(tpad, smax, lgroups=lgroups)
    nc = _CACHE[key]

    shared = _prep_shared(aggr_w, w1, b1, w2, b2, w3)
    shared["idp"] = np.eye(smax, dtype=np.float32)
    in_maps = []
    for c in range(NCORES):
        xt8 = np.zeros((12, tpad, F), dtype=f8)
        mkf = np.zeros((tpad, smax), dtype=np.float32)
        for j, (b, s0, s1, d0) in enumerate(segs[c]):
            n = s1 - s0
            xt8[:, d0:d0 + n] = x[b, :, s0:s1].astype(f8)
            mkf[d0:d0 + n, j] = 1.0 / float(lens[b])
        xq = np.ascontiguousarray(
            xt8.reshape(12, nsub, 12, G, F).transpose(1, 3, 0, 2, 4)
        ).reshape(nsub, SUB, 12, F)
        mk = np.ascontiguousarray(
            mkf.reshape(nsub, SUB, smax).transpose(1, 0, 2))
        in_maps.append({"x": xq, "maskw": mk, **shared})

    res = run_bass_kernel_spmd(nc, in_maps, core_ids=list(range(NCORES)))
    LAST_RESULTS = res
    LAST_META = (tpad, smax, in_maps)
    global LAST_LGROUPS
    LAST_LGROUPS = lgroups

    out = np.zeros((B, NL), dtype=np.float32)
    for c in range(NCORES):
        o = res.results[c]["out"]        # [NL, smax]
        for j, (b, s0, s1, d0) in enumerate(segs[c]):
            out[b] += o[:, j]
    out += b3[None, :]
    return out
